# revision 14
# baseline (speedup 1.0000x reference)
"""LocalAttentionBlock on 8 trn2 cores — bf16 rewrite.

Sharding: 8 cores = 2 batches x 4 sequence blocks of 512 queries, each
core sees a zero-padded 1024-wide context window (block +/- 256).

All matmul operands bf16 (f32 psum accumulation).  Per-core pipeline:
  kv projection fused into one stationary ([v|k] columns) so vT lands on
  psum partitions 0:64 and kT on 64:128; kT is copied to partitions 0:64
  of a second tile by an SBUF->SBUF DMA so even heads (row group 0:64)
  and odd heads (64:128) can run their S matmuls concurrently in
  disjoint PE row groups.  S^T is computed per (head, 256-query half) at
  band-tile granularity into a 3-bank psum tensor laid out as
  [lo-tri x2 | hi-tri x2 | full x6] so the exact |i-j|<=256 window cut
  is ONE 512-wide DVE multiply with a constant [mlo|mlo|mhi|mhi] tile
  after the exp (ACT, scale=1/8, psum->sbuf bf16).
  AV uses the ex tiles as the STATIONARY operand and va = [v^T | valid]
  (65 cols) as the moving operand, so the output lands [q, 64+1] with
  the softmax denominator in column 64: the normalization is then a
  per-partition reciprocal_approx_fast + tensor_scalar_mul (no
  partition broadcasts, no wide reciprocals).  Normalized attn pairs
  [128q, 128(2 heads)] are PE-transposed back to [feat, q] into the
  pad columns of the AV psum bank, evacuated to bf16, and fed to the
  final Wf matmul.  y is written bf16 and upcast on host.
"""
import sys

import numpy as np

sys.path.insert(0, "/opt/trn_rl_repo")

import ml_dtypes  # noqa: E402

import concourse.bass as bass  # noqa: E402,F401
import concourse.mybir as mybir  # noqa: E402
import concourse.tile as tile  # noqa: E402
from concourse import bacc  # noqa: E402
from concourse.bass import ts  # noqa: E402
from concourse.bass_utils import run_bass_kernel_spmd  # noqa: E402

F32 = mybir.dt.float32
BF16 = mybir.dt.bfloat16
AF = mybir.ActivationFunctionType
NPBF16 = ml_dtypes.bfloat16

B, T, D = 2, 2048, 1024
NH, HD = 16, 64
WIN = 256
BLK = 512      # queries per core
CTX = 1024     # padded context width
NCORES = 8

# S band pieces per (head, half): (jt_off, q_off, width, psum_col).
# jt = gb + jt_off (gb = 2*half), q_off is within the 256-query half.
# psum layout per half: [lo g0 | lo g1 | hi g0 | hi g1 | 128f | 256f |
# 128f | 256f] = 1280 used of 1536 (3 banks); no piece crosses a bank.
S_PIECES = [
    (0, 0, 128, 0),       # lo-tri a0
    (1, 0, 256, 128),     # full a0 + lo-tri a1
    (2, 0, 256, 768),     # full a0:a1
    (3, 0, 256, 1024),    # full a0:a1
    (4, 0, 256, 512),     # hi-tri a0 + full a1
    (5, 128, 128, 384),   # hi-tri a1
]
# start flag = first write into each 2KB psum bank, in issue order
S_START = [True, False, True, True, False, False]
# window cut: one DVE multiply over cols 0:640 with this column layout
# (mask constant = [mlo | ones | mlo | mhi | mhi])
MASK_W = 640
# ex tile column for AV stationary, [a][k]: head's q-tile a, s-tile g+k
AV_COLS = [
    [0, 128, 768, 1024, 512],
    [256, 896, 1152, 640, 384],
]
# issue chunk-1 (cols < 640) stationaries first within each AV chain
AV_ORDER = [
    [0, 1, 4, 2, 3],
    [0, 4, 3, 1, 2],
]


def _build():
    nc = bacc.Bacc(None)
    xT = nc.dram_tensor("xT", [D, CTX], BF16, kind="ExternalInput")
    wq = nc.dram_tensor("wq", [128, 8 * D], BF16, kind="ExternalInput")
    wkv = nc.dram_tensor("wkv", [128, 8 * 128], BF16, kind="ExternalInput")
    wf = nc.dram_tensor("wf", [128, 8 * D], BF16, kind="ExternalInput")
    bfin = nc.dram_tensor("bfin", [128, 8], F32, kind="ExternalInput")
    valid = nc.dram_tensor("valid", [128, 8], BF16, kind="ExternalInput")
    maskc = nc.dram_tensor("maskc", [128, 640], BF16, kind="ExternalInput")
    identc = nc.dram_tensor("identc", [128, 128], BF16, kind="ExternalInput")
    idf32c = nc.dram_tensor("idf32c", [128, 128], F32, kind="ExternalInput")
    yT = nc.dram_tensor("yT", [D, BLK], BF16, kind="ExternalOutput")

    with tile.TileContext(nc) as tc:
        with (
            tc.tile_pool(name="big", bufs=1) as big,
            tc.tile_pool(name="sm", bufs=1) as sm,
            tc.tile_pool(name="expp", bufs=2) as expp,
            tc.tile_pool(name="smd", bufs=2) as smd,
        ):
            # ---- input DMAs (small + kv first, then x, then wq) ----
            wkv_sb = big.tile([128, 8, 128], BF16, tag="wkv")
            nc.sync.dma_start(out=wkv_sb, in_=wkv.rearrange(
                "p (dt c) -> p dt c", c=128))
            ident = sm.tile([128, 128], BF16, tag="ident")
            nc.sync.dma_start(out=ident, in_=identc[:, :])
            idf32 = sm.tile([128, 128], F32, tag="idf32")
            nc.sync.dma_start(out=idf32, in_=idf32c[:, :])
            maskt = sm.tile([128, 640], BF16, tag="maskt")
            nc.sync.dma_start(out=maskt, in_=maskc[:, :])
            valid_sb = sm.tile([128, 8], BF16, tag="valid")
            nc.sync.dma_start(out=valid_sb, in_=valid[:, :])
            bf_sb = sm.tile([128, 8], F32, tag="bf")
            nc.sync.dma_start(out=bf_sb, in_=bfin[:, :])
            xt = big.tile([128, 8, CTX], BF16, tag="xt")
            for dt in range(8):
                nc.sync.dma_start(out=xt[:, dt, :], in_=xT[ts(dt, 128), :])
            wq_sb = big.tile([128, 8, D], BF16, tag="wq")
            for m in range(8):
                nc.sync.dma_start(out=wq_sb[:, m, :],
                                  in_=wq[:, ts(m, D)])
            wf_sb = big.tile([128, 8, D], BF16, tag="wf")
            for o in range(8):
                nc.sync.dma_start(out=wf_sb[:, o, :],
                                  in_=wf[:, ts(o, D)])

            kv_sb = big.tile([128, CTX], BF16, tag="kv")   # v 0:64, k 64:128
            k2 = big.tile([64, CTX], BF16, tag="k2")       # kT for even heads
            va = big.tile([128, 8, 65], BF16, tag="va")
            qT = big.tile([128, 8, BLK], BF16, tag="qT")
            anrm = big.tile([128, 8, BLK], BF16, tag="anrm")
            # normalized attn pairs [q, 2-head feat], one slot per
            # (pair, q-tile); transposed to [feat, q] in the tail
            apst = big.tile([128, 8, 4, 128], BF16, tag="apst")

            # ---- projections ----
            with (
                tc.tile_pool(name="psP", bufs=2, space="PSUM") as psP,
                tc.tile_pool(name="psV", bufs=2, space="PSUM") as psV,
            ):
                kv_ps = [psP.tile([128, 512], F32, tag="P",
                                  name=f"kv_ps{ch}") for ch in range(2)]
                for dt in range(8):
                    for ch in range(2):
                        nc.tensor.matmul(kv_ps[ch], wkv_sb[:, dt, :],
                                         xt[:, dt, ts(ch, 512)],
                                         start=(dt == 0), stop=(dt == 7))
                for ch in range(2):
                    (nc.vector.tensor_copy if ch == 0 else nc.scalar.copy)(
                        kv_sb[:, ts(ch, 512)], kv_ps[ch])
                # kT to partitions 0:64 for even heads (sbuf->sbuf DMA)
                nc.sync.dma_start(out=k2, in_=kv_sb[64:128, :])
                # v^T -> [s, h] tiles + valid column
                for jt in range(8):
                    t_ps = psV.tile([128, 64], BF16, tag="V",
                                    name=f"t_ps{jt}")
                    nc.tensor.transpose(t_ps, kv_sb[0:64, ts(jt, 128)],
                                        ident[0:64, 0:64])
                    nc.vector.tensor_copy(va[:, jt, 0:64], t_ps)
                nc.vector.tensor_copy(va[:, :, 64:65], valid_sb[:, :])
                for m in range(2):
                    q_ps = psP.tile([128, 512], F32, tag="P",
                                    name=f"q_ps{m}")
                    for dt in range(8):
                        nc.tensor.matmul(q_ps, wq_sb[:, m, ts(dt, 128)],
                                         xt[:, dt, 256:768],
                                         start=(dt == 0), stop=(dt == 7))
                    (nc.vector.tensor_copy if m % 2 else nc.scalar.copy)(
                        qT[:, m, :], q_ps)

            # ---- attention middle ----
            with (
                tc.tile_pool(name="psA", bufs=1, space="PSUM") as psA,
                tc.tile_pool(name="psB", bufs=1, space="PSUM") as psB,
                tc.tile_pool(name="psC", bufs=2, space="PSUM") as psC,
            ):
                for m in range(8):
                    for hf in range(2):
                        gb = 2 * hf
                        qb = 256 * hf
                        # S^T for even/odd head of the pair, concurrent
                        # PE row groups (kT/qT partitions 0:64 vs 64:128)
                        sps = [psA.tile([128, 1536], F32, tag="S0",
                                        name=f"s_ps0_{m}_{hf}"),
                               psB.tile([128, 1536], F32, tag="S1",
                                        name=f"s_ps1_{m}_{hf}")]
                        for odd in range(2):
                            r0 = 64 * odd
                            kT = kv_sb[64:128, :] if odd else k2[0:64, :]
                            for (jo, qo, w, col), st in zip(S_PIECES,
                                                            S_START):
                                nc.tensor.matmul(
                                    sps[odd][:, col:col + w],
                                    kT[:, ts(gb + jo, 128)],
                                    qT[r0:r0 + 64, m, qb + qo:qb + qo + w],
                                    start=st, stop=True,
                                    skip_group_check=True)
                        exs = []
                        for odd in range(2):
                            ex = expp.tile([128, 1280], BF16,
                                           tag=f"ex{odd}")
                            # two chunks: the mask and the first AV
                            # matmuls only need cols 0:640
                            nc.scalar.activation(out=ex[:, 0:MASK_W],
                                                 in_=sps[odd][:, 0:MASK_W],
                                                 func=AF.Exp, scale=0.125)
                            nc.vector.tensor_mul(ex[:, 0:MASK_W],
                                                 ex[:, 0:MASK_W], maskt)
                            nc.scalar.activation(
                                out=ex[:, MASK_W:1280],
                                in_=sps[odd][:, MASK_W:1280],
                                func=AF.Exp, scale=0.125)
                            exs.append(ex)
                        # AV: ex tiles stationary, va moving ->
                        # out [q, 64 attn + denom]; both heads' 4 chains
                        # share ONE psum bank so psC double-buffers at
                        # half-pair granularity (tile frees after norm)
                        c_ps = psC.tile([128, 512], F32, tag="C",
                                        name=f"c_ps_{m}_{hf}")
                        for odd in range(2):
                            for a in range(2):
                                base = 256 * odd + 128 * a
                                order = AV_ORDER[a]
                                for i, k in enumerate(order):
                                    col = AV_COLS[a][k]
                                    nc.tensor.matmul(
                                        c_ps[:, base:base + 65],
                                        exs[odd][:, col:col + 128],
                                        va[:, gb + a + k, :],
                                        start=(odd == 0 and a == 0
                                               and i == 0),
                                        stop=(i == 4),
                                        skip_group_check=True)
                        # normalize: per-partition denom -> recip ->
                        # scale, into persistent [q, feat-pair] slots
                        rec = smd.tile([128, 4], F32, tag="rec")
                        nc.vector.reciprocal_approx_fast(
                            out=rec,
                            in_=c_ps.rearrange(
                                "p (s c) -> p s c", c=128)[:, :, 64])
                        for odd in range(2):
                            for a in range(2):
                                base = 256 * odd + 128 * a
                                nc.vector.tensor_scalar_mul(
                                    apst[:, m, gb + a,
                                         64 * odd:64 * odd + 64],
                                    c_ps[:, base:base + 64],
                                    rec[:, 2 * odd + a:2 * odd + a + 1])
                        # deferred q projections: dense 512-wide
                        # matmuls fill PE gaps while ACT/DVE work the
                        # softmax; front-loaded where the pipeline is
                        # still filling
                        if hf == 1 and m + 2 < 8:
                            mq = m + 2
                            q_ps = psC.tile([128, 512], F32, tag="C",
                                            name=f"q_ps{mq}")
                            for dt in range(8):
                                nc.tensor.matmul(
                                    q_ps, wq_sb[:, mq, ts(dt, 128)],
                                    xt[:, dt, 256:768],
                                    start=(dt == 0), stop=(dt == 7))
                            (nc.vector.tensor_copy if mq % 2 else
                             nc.scalar.copy)(qT[:, mq, :], q_ps)

            # ---- tail: transpose attn pairs (bf16), then Wf ----
            with (
                tc.tile_pool(name="psT", bufs=2, space="PSUM") as psT,
                tc.tile_pool(name="psY", bufs=2, space="PSUM") as psY,
            ):
                for m in range(8):
                    t_ps = psT.tile([128, 512], BF16, tag="T",
                                    name=f"tr_{m}")
                    for g in range(4):
                        nc.tensor.matmul(
                            t_ps[:, ts(g, 128)], apst[:, m, g, :],
                            ident, is_transpose=True,
                            start=(g == 0), stop=True,
                            skip_group_check=True)
                    (nc.scalar.copy if m % 2 else
                     nc.vector.tensor_copy)(anrm[:, m, :], t_ps)
                for o in range(8):
                    y_ps = psY.tile([128, 512], F32, tag="Y",
                                    name=f"y_ps{o}")
                    for ft in range(8):
                        nc.tensor.matmul(y_ps, wf_sb[:, o, ts(ft, 128)],
                                         anrm[:, ft, :],
                                         start=(ft == 0), stop=(ft == 7))
                    y_sb = big.tile([128, BLK], BF16, tag=f"y{o % 2}",
                                    name=f"y_sb{o}")
                    nc.vector.tensor_scalar_add(y_sb, y_ps,
                                                bf_sb[:, o:o + 1])
                    nc.sync.dma_start(out=yT[ts(o, 128), :], in_=y_sb)

    nc.compile()
    return nc


_NC = None


def _get_nc():
    global _NC
    if _NC is None:
        _NC = _build()
    return _NC


def _prep_inputs(x, Wq, Wk, Wv, Wf, bf):
    x = np.asarray(x, np.float32)
    Wq = np.asarray(Wq, np.float32)
    Wk = np.asarray(Wk, np.float32)
    Wv = np.asarray(Wv, np.float32)
    Wf = np.asarray(Wf, np.float32)
    # wkv stationary per dt: cols 0:64 -> v rows, 64:128 -> k rows
    wkv_h = np.empty((128, 8, 128), np.float32)
    wq_h = np.empty((128, 8, 8, 128), np.float32)
    wf_h = np.empty((128, 8, 8, 128), np.float32)
    for dt in range(8):
        rows = slice(128 * dt, 128 * dt + 128)
        wkv_h[:, dt, 0:64] = Wv[:, rows].T
        wkv_h[:, dt, 64:128] = Wk[:, rows].T
        for m in range(8):
            wq_h[:, m, dt, :] = Wq[ts_np(m), rows.start:rows.stop].T
            wf_h[:, m, dt, :] = Wf[ts_np(m), rows.start:rows.stop].T
    u = np.arange(128, dtype=np.float32)
    mlo = (u[:, None] >= u[None, :]).astype(np.float32)
    mhi = (u[:, None] <= u[None, :]).astype(np.float32)
    ones = np.ones((128, 128), np.float32)
    maskc = np.concatenate([mlo, ones, mlo, mhi, mhi], axis=1)
    ident = np.eye(128, dtype=np.float32)
    shared = {
        "wq": np.ascontiguousarray(
            wq_h.reshape(128, 8192)).astype(NPBF16),
        "wkv": np.ascontiguousarray(
            wkv_h.reshape(128, 1024)).astype(NPBF16),
        "wf": np.ascontiguousarray(
            wf_h.reshape(128, 8192)).astype(NPBF16),
        "bfin": np.ascontiguousarray(
            np.asarray(bf, np.float32).reshape(8, 128).T),
        "maskc": maskc.astype(NPBF16),
        "identc": ident.astype(NPBF16),
        "idf32c": ident,
    }
    in_maps = []
    for c in range(NCORES):
        b, i = divmod(c, 4)
        g0 = 512 * i - WIN  # global position of ctx col 0
        xTc = np.zeros((D, CTX), np.float32)
        lo, hi = max(0, g0), min(T, g0 + CTX)
        xTc[:, lo - g0:hi - g0] = x[b, lo:hi, :].T
        s = np.arange(CTX)
        vmask = ((s + g0 >= 0) & (s + g0 < T)).astype(np.float32)
        in_maps.append({
            "xT": xTc.astype(NPBF16),
            "valid": np.ascontiguousarray(
                vmask.reshape(8, 128).T).astype(NPBF16),
            **shared,
        })
    return in_maps


def ts_np(i, size=128):
    return slice(size * i, size * (i + 1))


def _run(inputs, trace=False):
    nc = _get_nc()
    in_maps = _prep_inputs(**inputs)
    res = run_bass_kernel_spmd(nc, in_maps, core_ids=list(range(NCORES)),
                               trace=trace)
    x = inputs["x"]
    out = np.empty((B, T, D), np.float32)
    for c in range(NCORES):
        b, i = divmod(c, 4)
        out[b, 512 * i:512 * (i + 1), :] = \
            res.results[c]["yT"].astype(np.float32).T
    return out.astype(np.asarray(x).dtype), res


def kernel(**inputs):
    out, _ = _run(inputs)
    return out


# revision 15
# speedup vs baseline: 1.2624x; 1.2624x over previous
"""LocalAttentionBlock on 8 trn2 cores — bf16 rewrite.

Sharding: 8 cores = 2 batches x 4 sequence blocks of 512 queries, each
core sees a zero-padded 1024-wide context window (block +/- 256).

All matmul operands bf16 (f32 psum accumulation).  Per-core pipeline:
  kv projection fused into one stationary ([v|k] columns) so vT lands on
  psum partitions 0:64 and kT on 64:128; kT is copied to partitions 0:64
  of a second tile by an SBUF->SBUF DMA so even heads (row group 0:64)
  and odd heads (64:128) can run their S matmuls concurrently in
  disjoint PE row groups.  S^T is computed per (head, 256-query half) at
  band-tile granularity into a 3-bank psum tensor laid out as
  [lo-tri x2 | hi-tri x2 | full x6] so the exact |i-j|<=256 window cut
  is ONE 512-wide DVE multiply with a constant [mlo|mlo|mhi|mhi] tile
  after the exp (ACT, scale=1/8, psum->sbuf bf16).
  AV uses the ex tiles as the STATIONARY operand and va = [v^T | valid]
  (65 cols) as the moving operand, so the output lands [q, 64+1] with
  the softmax denominator in column 64: the normalization is then a
  per-partition reciprocal_approx_fast + tensor_scalar_mul (no
  partition broadcasts, no wide reciprocals).  Normalized attn pairs
  [128q, 128(2 heads)] are PE-transposed back to [feat, q] into the
  pad columns of the AV psum bank, evacuated to bf16, and fed to the
  final Wf matmul.  y is written bf16 and upcast on host.
"""
import sys

import numpy as np

sys.path.insert(0, "/opt/trn_rl_repo")

import ml_dtypes  # noqa: E402

import concourse.bass as bass  # noqa: E402,F401
import concourse.mybir as mybir  # noqa: E402
import concourse.tile as tile  # noqa: E402
from concourse import bacc  # noqa: E402
from concourse.bass import ts  # noqa: E402
from concourse.bass_utils import run_bass_kernel_spmd  # noqa: E402

F32 = mybir.dt.float32
BF16 = mybir.dt.bfloat16
AF = mybir.ActivationFunctionType
NPBF16 = ml_dtypes.bfloat16

B, T, D = 2, 2048, 1024
NH, HD = 16, 64
WIN = 256
BLK = 512      # queries per core
CTX = 1024     # padded context width
NCORES = 8

# S band pieces per (head, half): (jt_off, q_off, width, psum_col).
# jt = gb + jt_off (gb = 2*half), q_off is within the 256-query half.
# psum layout per half: [lo g0 | lo g1 | hi g0 | hi g1 | 128f | 256f |
# 128f | 256f] = 1280 used of 1536 (3 banks); no piece crosses a bank.
S_PIECES = [
    (0, 0, 128, 0),       # lo-tri a0
    (1, 0, 256, 128),     # full a0 + lo-tri a1
    (2, 0, 256, 768),     # full a0:a1
    (3, 0, 256, 1024),    # full a0:a1
    (4, 0, 256, 512),     # hi-tri a0 + full a1
    (5, 128, 128, 384),   # hi-tri a1
]
# start flag = first write into each 2KB psum bank, in issue order
S_START = [True, False, True, True, False, False]
# window cut: one DVE multiply over cols 0:640 with this column layout
# (mask constant = [mlo | ones | mlo | mhi | mhi])
MASK_W = 640
# ex tile column for AV stationary, [a][k]: head's q-tile a, s-tile g+k
AV_COLS = [
    [0, 128, 768, 1024, 512],
    [256, 896, 1152, 640, 384],
]
# issue chunk-1 (cols < 640) stationaries first within each AV chain
AV_ORDER = [
    [0, 1, 4, 2, 3],
    [0, 4, 3, 1, 2],
]


def _build():
    nc = bacc.Bacc(None)
    xT = nc.dram_tensor("xT", [D, CTX], BF16, kind="ExternalInput")
    wq = nc.dram_tensor("wq", [128, 8 * D], BF16, kind="ExternalInput")
    wkv = nc.dram_tensor("wkv", [128, 8 * 128], BF16, kind="ExternalInput")
    wf = nc.dram_tensor("wf", [128, 8 * D], BF16, kind="ExternalInput")
    bfin = nc.dram_tensor("bfin", [128, 8], F32, kind="ExternalInput")
    valid = nc.dram_tensor("valid", [128, 8], BF16, kind="ExternalInput")
    maskc = nc.dram_tensor("maskc", [128, 640], BF16, kind="ExternalInput")
    identc = nc.dram_tensor("identc", [128, 128], BF16, kind="ExternalInput")
    idf32c = nc.dram_tensor("idf32c", [128, 128], F32, kind="ExternalInput")
    yT = nc.dram_tensor("yT", [D, BLK], BF16, kind="ExternalOutput")

    with tile.TileContext(nc) as tc:
        with (
            tc.tile_pool(name="big", bufs=1) as big,
            tc.tile_pool(name="sm", bufs=1) as sm,
            tc.tile_pool(name="expp", bufs=2) as expp,
            tc.tile_pool(name="smd", bufs=2) as smd,
        ):
            # ---- input DMAs (small + kv first, then x, then wq) ----
            wkv_sb = big.tile([128, 8, 128], BF16, tag="wkv")
            nc.sync.dma_start(out=wkv_sb, in_=wkv.rearrange(
                "p (dt c) -> p dt c", c=128))
            ident = sm.tile([128, 128], BF16, tag="ident")
            nc.sync.dma_start(out=ident, in_=identc[:, :])
            idf32 = sm.tile([128, 128], F32, tag="idf32")
            nc.sync.dma_start(out=idf32, in_=idf32c[:, :])
            maskt = sm.tile([128, 640], BF16, tag="maskt")
            nc.sync.dma_start(out=maskt, in_=maskc[:, :])
            valid_sb = sm.tile([128, 8], BF16, tag="valid")
            nc.sync.dma_start(out=valid_sb, in_=valid[:, :])
            bf_sb = sm.tile([128, 8], F32, tag="bf")
            nc.sync.dma_start(out=bf_sb, in_=bfin[:, :])
            xt = big.tile([128, 8, CTX], BF16, tag="xt")
            for dt in range(8):
                for h in range(2):
                    nc.sync.dma_start(
                        out=xt[64 * h:64 * h + 64, dt, :],
                        in_=xT[128 * dt + 64 * h:128 * dt + 64 * h + 64,
                               :])
            wq_sb = big.tile([128, 8, D], BF16, tag="wq")
            for m in range(8):
                for h in range(2):
                    nc.sync.dma_start(
                        out=wq_sb[64 * h:64 * h + 64, m, :],
                        in_=wq[64 * h:64 * h + 64, ts(m, D)])
            wf_sb = big.tile([128, 8, D], BF16, tag="wf")
            for o in range(8):
                for h in range(2):
                    nc.sync.dma_start(
                        out=wf_sb[64 * h:64 * h + 64, o, :],
                        in_=wf[64 * h:64 * h + 64, ts(o, D)])

            kv_sb = big.tile([128, CTX], BF16, tag="kv")   # v 0:64, k 64:128
            k2 = big.tile([64, CTX], BF16, tag="k2")       # kT for even heads
            va = big.tile([128, 8, 65], BF16, tag="va")
            qT = big.tile([128, 8, BLK], BF16, tag="qT")
            anrm = big.tile([128, 8, BLK], BF16, tag="anrm")
            # normalized attn pairs [q, 2-head feat], one slot per
            # (pair, q-tile); transposed to [feat, q] in the tail
            apst = big.tile([128, 8, 4, 128], BF16, tag="apst")

            # ---- projections ----
            with (
                tc.tile_pool(name="psP", bufs=2, space="PSUM") as psP,
                tc.tile_pool(name="psV", bufs=2, space="PSUM") as psV,
            ):
                kv_ps = [psP.tile([128, 512], F32, tag="P",
                                  name=f"kv_ps{ch}") for ch in range(2)]
                for dt in range(8):
                    for ch in range(2):
                        nc.tensor.matmul(kv_ps[ch], wkv_sb[:, dt, :],
                                         xt[:, dt, ts(ch, 512)],
                                         start=(dt == 0), stop=(dt == 7))
                for ch in range(2):
                    (nc.vector.tensor_copy if ch == 0 else nc.scalar.copy)(
                        kv_sb[:, ts(ch, 512)], kv_ps[ch])
                # kT to partitions 0:64 for even heads (sbuf->sbuf DMA)
                nc.sync.dma_start(out=k2, in_=kv_sb[64:128, :])
                # v^T -> [s, h] tiles + valid column
                for jt in range(8):
                    t_ps = psV.tile([128, 64], BF16, tag="V",
                                    name=f"t_ps{jt}")
                    nc.tensor.transpose(t_ps, kv_sb[0:64, ts(jt, 128)],
                                        ident[0:64, 0:64])
                    nc.vector.tensor_copy(va[:, jt, 0:64], t_ps)
                nc.vector.tensor_copy(va[:, :, 64:65], valid_sb[:, :])
                for m in range(2):
                    q_ps = psP.tile([128, 512], F32, tag="P",
                                    name=f"q_ps{m}")
                    for dt in range(8):
                        nc.tensor.matmul(q_ps, wq_sb[:, m, ts(dt, 128)],
                                         xt[:, dt, 256:768],
                                         start=(dt == 0), stop=(dt == 7))
                    (nc.vector.tensor_copy if m % 2 else nc.scalar.copy)(
                        qT[:, m, :], q_ps)

            # ---- attention middle ----
            with (
                tc.tile_pool(name="psA", bufs=1, space="PSUM") as psA,
                tc.tile_pool(name="psB", bufs=1, space="PSUM") as psB,
                tc.tile_pool(name="psC", bufs=2, space="PSUM") as psC,
            ):
                for m in range(8):
                    for hf in range(2):
                        gb = 2 * hf
                        qb = 256 * hf
                        # S^T for even/odd head of the pair, concurrent
                        # PE row groups (kT/qT partitions 0:64 vs 64:128)
                        sps = [psA.tile([128, 1536], F32, tag="S0",
                                        name=f"s_ps0_{m}_{hf}"),
                               psB.tile([128, 1536], F32, tag="S1",
                                        name=f"s_ps1_{m}_{hf}")]
                        for odd in range(2):
                            r0 = 64 * odd
                            kT = kv_sb[64:128, :] if odd else k2[0:64, :]
                            for (jo, qo, w, col), st in zip(S_PIECES,
                                                            S_START):
                                nc.tensor.matmul(
                                    sps[odd][:, col:col + w],
                                    kT[:, ts(gb + jo, 128)],
                                    qT[r0:r0 + 64, m, qb + qo:qb + qo + w],
                                    start=st, stop=True,
                                    skip_group_check=True)
                        exs = []
                        for odd in range(2):
                            ex = expp.tile([128, 1280], BF16,
                                           tag=f"ex{odd}")
                            nc.scalar.activation(out=ex,
                                                 in_=sps[odd][:, 0:1280],
                                                 func=AF.Exp, scale=0.125)
                            # exact window cut on the 8 diagonal tiles
                            nc.vector.tensor_mul(ex[:, 0:MASK_W],
                                                 ex[:, 0:MASK_W], maskt)
                            exs.append(ex)
                        # AV: ex tiles stationary, va moving ->
                        # out [q, 64 attn + denom]; both heads' 4 chains
                        # share ONE psum bank so psC double-buffers at
                        # half-pair granularity (tile frees after norm)
                        c_ps = psC.tile([128, 512], F32, tag="C",
                                        name=f"c_ps_{m}_{hf}")
                        for odd in range(2):
                            for a in range(2):
                                base = 256 * odd + 128 * a
                                for k in range(5):
                                    col = AV_COLS[a][k]
                                    nc.tensor.matmul(
                                        c_ps[:, base:base + 65],
                                        exs[odd][:, col:col + 128],
                                        va[:, gb + a + k, :],
                                        start=(odd == 0 and a == 0
                                               and k == 0),
                                        stop=(k == 4),
                                        skip_group_check=True)
                        # normalize: per-partition denom -> recip ->
                        # scale, into persistent [q, feat-pair] slots
                        rec = smd.tile([128, 4], F32, tag="rec")
                        nc.vector.reciprocal_approx_fast(
                            out=rec,
                            in_=c_ps.rearrange(
                                "p (s c) -> p s c", c=128)[:, :, 64])
                        for odd in range(2):
                            for a in range(2):
                                base = 256 * odd + 128 * a
                                nc.vector.tensor_scalar_mul(
                                    apst[:, m, gb + a,
                                         64 * odd:64 * odd + 64],
                                    c_ps[:, base:base + 64],
                                    rec[:, 2 * odd + a:2 * odd + a + 1])
                        # deferred q projections: dense 512-wide
                        # matmuls fill PE gaps while ACT/DVE work the
                        # softmax; front-loaded where the pipeline is
                        # still filling
                        if hf == 1 and m + 2 < 8:
                            mq = m + 2
                            q_ps = psC.tile([128, 512], F32, tag="C",
                                            name=f"q_ps{mq}")
                            for dt in range(8):
                                nc.tensor.matmul(
                                    q_ps, wq_sb[:, mq, ts(dt, 128)],
                                    xt[:, dt, 256:768],
                                    start=(dt == 0), stop=(dt == 7))
                            (nc.vector.tensor_copy if mq % 2 else
                             nc.scalar.copy)(qT[:, mq, :], q_ps)

            # ---- tail: transpose attn pairs (bf16), then Wf ----
            with (
                tc.tile_pool(name="psT", bufs=2, space="PSUM") as psT,
                tc.tile_pool(name="psY", bufs=2, space="PSUM") as psY,
            ):
                for m in range(8):
                    t_ps = psT.tile([128, 512], BF16, tag="T",
                                    name=f"tr_{m}")
                    for g in range(4):
                        nc.tensor.matmul(
                            t_ps[:, ts(g, 128)], apst[:, m, g, :],
                            ident, is_transpose=True,
                            start=(g == 0), stop=True,
                            skip_group_check=True)
                    (nc.scalar.copy if m % 2 else
                     nc.vector.tensor_copy)(anrm[:, m, :], t_ps)
                for o in range(8):
                    y_ps = psY.tile([128, 512], F32, tag="Y",
                                    name=f"y_ps{o}")
                    for ft in range(8):
                        nc.tensor.matmul(y_ps, wf_sb[:, o, ts(ft, 128)],
                                         anrm[:, ft, :],
                                         start=(ft == 0), stop=(ft == 7))
                    y_sb = big.tile([128, BLK], BF16, tag=f"y{o % 2}",
                                    name=f"y_sb{o}")
                    nc.vector.tensor_scalar_add(y_sb, y_ps,
                                                bf_sb[:, o:o + 1])
                    for h in range(2):
                        nc.sync.dma_start(
                            out=yT[128 * o + 64 * h:
                                   128 * o + 64 * h + 64, :],
                            in_=y_sb[64 * h:64 * h + 64, :])

    nc.compile()
    return nc


_NC = None


def _get_nc():
    global _NC
    if _NC is None:
        _NC = _build()
    return _NC


def _prep_inputs(x, Wq, Wk, Wv, Wf, bf):
    x = np.asarray(x, np.float32)
    Wq = np.asarray(Wq, np.float32)
    Wk = np.asarray(Wk, np.float32)
    Wv = np.asarray(Wv, np.float32)
    Wf = np.asarray(Wf, np.float32)
    # wkv stationary per dt: cols 0:64 -> v rows, 64:128 -> k rows
    wkv_h = np.empty((128, 8, 128), np.float32)
    wq_h = np.empty((128, 8, 8, 128), np.float32)
    wf_h = np.empty((128, 8, 8, 128), np.float32)
    for dt in range(8):
        rows = slice(128 * dt, 128 * dt + 128)
        wkv_h[:, dt, 0:64] = Wv[:, rows].T
        wkv_h[:, dt, 64:128] = Wk[:, rows].T
        for m in range(8):
            wq_h[:, m, dt, :] = Wq[ts_np(m), rows.start:rows.stop].T
            wf_h[:, m, dt, :] = Wf[ts_np(m), rows.start:rows.stop].T
    u = np.arange(128, dtype=np.float32)
    mlo = (u[:, None] >= u[None, :]).astype(np.float32)
    mhi = (u[:, None] <= u[None, :]).astype(np.float32)
    ones = np.ones((128, 128), np.float32)
    maskc = np.concatenate([mlo, ones, mlo, mhi, mhi], axis=1)
    ident = np.eye(128, dtype=np.float32)
    shared = {
        "wq": np.ascontiguousarray(
            wq_h.reshape(128, 8192)).astype(NPBF16),
        "wkv": np.ascontiguousarray(
            wkv_h.reshape(128, 1024)).astype(NPBF16),
        "wf": np.ascontiguousarray(
            wf_h.reshape(128, 8192)).astype(NPBF16),
        "bfin": np.ascontiguousarray(
            np.asarray(bf, np.float32).reshape(8, 128).T),
        "maskc": maskc.astype(NPBF16),
        "identc": ident.astype(NPBF16),
        "idf32c": ident,
    }
    in_maps = []
    for c in range(NCORES):
        b, i = divmod(c, 4)
        g0 = 512 * i - WIN  # global position of ctx col 0
        xTc = np.zeros((D, CTX), np.float32)
        lo, hi = max(0, g0), min(T, g0 + CTX)
        xTc[:, lo - g0:hi - g0] = x[b, lo:hi, :].T
        s = np.arange(CTX)
        vmask = ((s + g0 >= 0) & (s + g0 < T)).astype(np.float32)
        in_maps.append({
            "xT": xTc.astype(NPBF16),
            "valid": np.ascontiguousarray(
                vmask.reshape(8, 128).T).astype(NPBF16),
            **shared,
        })
    return in_maps


def ts_np(i, size=128):
    return slice(size * i, size * (i + 1))


def _run(inputs, trace=False):
    nc = _get_nc()
    in_maps = _prep_inputs(**inputs)
    res = run_bass_kernel_spmd(nc, in_maps, core_ids=list(range(NCORES)),
                               trace=trace)
    x = inputs["x"]
    out = np.empty((B, T, D), np.float32)
    for c in range(NCORES):
        b, i = divmod(c, 4)
        out[b, 512 * i:512 * (i + 1), :] = \
            res.results[c]["yT"].astype(np.float32).T
    return out.astype(np.asarray(x).dtype), res


def kernel(**inputs):
    out, _ = _run(inputs)
    return out
